# revision 1
# baseline (speedup 1.0000x reference)
"""Trainium2 Bass kernel for nn_DecisionMaker (retrieval_knn).

Strategy (v1): pure data-parallel SPMD over batch. B=128 is split into 8
slices of 16; every NeuronCore runs the identical program on its slice.
Weights are replicated (host pre-transposes them into the layouts the
tensor engine wants); embedding-table rows are fetched with dma_gather so
the 20MB tables never leave HBM.

Per-core layout: tb = t*16 + b_local (t-major), tb in [0, 336). Activations
live "feature-on-partition": [F (128-chunks), tb (free)], so every matmul
is out.T = Wt-chunks (lhsT) @ x.T (rhs) with contraction on the partition
axis and no on-device weight transposes.

GRU: per-core batch-16 recurrence, both directions. The recurrent matmul
uses h.T (bf16) as the stationary operand; h.T is regenerated each step
with 16x128 xbar DMA transposes that overlap PE work. Gate math packs both
directions into one [48, F] tile (rows 0:16 fwd, 32:48 bwd) so ACT ops are
shared; the sigmoid/tanh table set never switches inside the loop.
"""

import functools

import numpy as np
import ml_dtypes

import concourse.bass as bass
import concourse.tile as tile
from concourse import bacc, mybir
from concourse.bass_utils import run_bass_kernel_spmd

F32 = mybir.dt.float32
BF16 = mybir.dt.bfloat16
I16 = mybir.dt.int16
AF = mybir.ActivationFunctionType
OP = mybir.AluOpType
BF = ml_dtypes.bfloat16

NCORE = 8
B, T, K, E, H, IC = 128, 21, 6, 512, 512, 2048
CV = 10001
BPC = B // NCORE            # 16 batch rows per core
TB = T * BPC                # 336 (t-major: tb = t*16 + b)
TBP = 384                   # padded to 3*128 for gathers
NCH = 3                     # tb chunks of 128
G3 = 3 * H                  # 1536 gate width
PAIRS = [(i, j) for i in range(K) for j in range(i + 1, K)]  # 15, (0,k) first


def _emit(nc, tc, io, stage=99):
    from contextlib import ExitStack
    ctx = ExitStack()
    wp = ctx.enter_context(tc.tile_pool(name="weights", bufs=1))
    ws = ctx.enter_context(tc.tile_pool(name="wstream", bufs=3))
    ap_ = ctx.enter_context(tc.tile_pool(name="acts", bufs=1))
    gp = ctx.enter_context(tc.tile_pool(name="gather", bufs=1))
    sp = ctx.enter_context(tc.tile_pool(name="scratch", bufs=2))
    fp = ctx.enter_context(tc.tile_pool(name="feats", bufs=1))
    pm = ctx.enter_context(tc.tile_pool(name="psum_mm", bufs=1, space="PSUM"))
    pc = ctx.enter_context(tc.tile_pool(name="psum_cnn", bufs=1, space="PSUM"))
    pg = ctx.enter_context(tc.tile_pool(name="psum_gh", bufs=2, space="PSUM"))
    nrep = 8 if stage == 98 else 1
    for _rep in range(nrep):
        _emit_body(nc, tc, io, 99 if stage == 98 else stage, ctx,
                   wp, ws, ap_, gp, sp, fp, pm, pc, pg, close=(_rep == nrep - 1))


def _emit_body(nc, tc, io, stage, ctx, wp, ws, ap_, gp, sp, fp, pm, pc, pg,
               close=True):

    # ---------------- resident weights (bf16, pre-transposed on host)
    def load_w(name, shape, dt=BF16):
        t = wp.tile(list(shape), dt, tag=name)
        nc.sync.dma_start(t[:], io[name][:])
        return t

    def load_w_chunks(name, rows, cols, dt=BF16):
        ts = []
        for k in range(rows // 128):
            t = wp.tile([128, cols], dt, tag=f"{name}{k}")
            nc.sync.dma_start(t[:], io[name][k * 128:(k + 1) * 128, :])
            ts.append(t)
        return ts

    hWT = load_w_chunks("hWT", 512, 256)          # 4 x [128, 256]
    pWT = load_w("pWT", [128, 256])
    posW = load_w("posW", [51, 128])              # pos_emb_w, natural lhsT
    w1T = load_w("w1T", [12, 128])
    w2T = load_w_chunks("w2T", 384, 256)          # 3 x [128, 256]
    w3T = load_w_chunks("w3T", 512, 512)          # 4 x [128, 512]
    oWT = load_w_chunks("oWT", 1024, 1)           # 8 x [128, 1]
    wih = {d: load_w_chunks(f"wihT_{d}", 512, G3) for d in "fb"}
    whh = {d: load_w_chunks(f"whhT_{d}", 512, G3) for d in "fb"}

    def load_bias(name, f):
        p = min(f, 128)
        m = max(1, f // 128)
        t = wp.tile([p, m], F32, tag=name)
        src = io[name].rearrange("(m p) -> p m", p=p) if f > 128 \
            else io[name].unsqueeze(1)
        nc.sync.dma_start(t[:], src)
        return t

    b_ib = load_bias("ib", 512)
    b_pb = load_bias("pb", 256)
    b_hb = load_bias("hb", 256)
    b_cb = load_bias("cb", 512)
    b_c1 = load_bias("b1", 128)
    b_c2 = load_bias("b2", 256)
    b_c3 = load_bias("b3", 512)
    gbias = {d: load_w(f"gbias_{d}", [128, G3]) for d in "fb"}  # pre-broadcast
    ident = load_w("ident", [128, 128])           # bf16 identity (PE transpose)

    # ---------------- activations
    att = []
    for k in range(IC // 128):
        t = ap_.tile([128, TB], BF16, tag=f"att{k}")
        nc.sync.dma_start(t[:], io["attT"][k * 128:(k + 1) * 128, :])
        att.append(t)
    hid = []
    for k in range(4):
        t = ap_.tile([128, TB], BF16, tag=f"hid{k}")
        nc.sync.dma_start(t[:], io["hidT"][k * 128:(k + 1) * 128, :])
        hid.append(t)
    posT = ap_.tile([51, TB], BF16, tag="posT")
    nc.sync.dma_start(posT[:], io["posT"][:])
    probs = ap_.tile([128, NCH, K], F32, tag="probs")
    nc.sync.dma_start(probs[:], io["probsP"].rearrange("(c p) k -> p c k", p=128))
    sent = ap_.tile([128, E], BF16, tag="sent")   # pre-broadcast rows p -> b=p%16
    nc.sync.dma_start(sent[:], io["sentB"][:])

    if stage == 0:   # loads only
        lgd = fp.tile([1, TB], F32, tag="lg", name="lgdbg")
        nc.vector.memset(lgd[:], 0.0)
        nc.vector.tensor_copy(lgd[0:1, 0:TB], att[0][0:1, 0:TB])
        nc.sync.dma_start(io["out_logits"][:], lgd[:])
        ctx.close()
        return

    # ---------------- gathers (tables stay in DRAM)
    # gidx columns: 0..17 = emb (k, c); 18..20 = cap_f c; 21..23 = cap_b c
    gidx = gp.tile([128, 24], mybir.dt.int32, tag="gidx")
    nc.sync.dma_start(gidx[:], io["gidx"][:])

    def gather_rows(dst, table, col):
        nc.gpsimd.indirect_dma_start(
            out=dst, out_offset=None, in_=table,
            in_offset=bass.IndirectOffsetOnAxis(ap=gidx[:, col:col + 1], axis=0))

    emb = []
    for k in range(K):
        out = gp.tile([128, NCH, E], BF16, tag=f"emb{k}", name=f"emb{k}")
        for c in range(NCH):
            gather_rows(out[:, c, :], io["cap_emb_bf"][:], k * NCH + c)
        emb.append(out)

    capeT = {}
    for di, d in enumerate("fb"):
        nat = gp.tile([128, NCH, E], BF16, tag=f"capeN_{d}", name=f"capeN_{d}")
        for c in range(NCH):
            gather_rows(nat[:, c, :], io["cap_emb_w_bf"][:], 18 + di * NCH + c)
        out = gp.tile([128, 4, TBP], BF16, tag=f"capeT_{d}", name=f"capeT_{d}")
        for kk in range(4):
            for c in range(NCH):
                nc.sync.dma_start_transpose(
                    out[:, kk, c * 128:(c + 1) * 128],
                    nat[:, c, kk * 128:(kk + 1) * 128])
        capeT[d] = out

    def _dbg_out(src, n=TB):
        lgd = fp.tile([1, TB], F32, tag="lg", name="lgdbg")
        nc.vector.memset(lgd[:], 0.0)
        nc.vector.tensor_copy(lgd[0:1, 0:n], src)
        nc.sync.dma_start(io["out_logits"][:], lgd[:])
        ctx.close()

    if stage == 1:   # gathers only
        _dbg_out(emb[0][0:1, 0, 0:TB])
        return

    # ---------------- gi = cap_e @ w_ih.T (+ gru bias) -> gx tiles
    gx = {"f": [], "b": []}
    for d in "fb":
        for mc in range(NCH):
            ps = pg.tile([128, G3], F32, tag="gh_ps", name="gi_ps")
            for kk in range(4):
                lhsT = capeT[d][:, kk, mc * 128:(mc + 1) * 128]
                for nb in range(3):
                    nc.tensor.matmul(
                        ps[:, nb * 512:(nb + 1) * 512],
                        lhsT, wih[d][kk][:, nb * 512:(nb + 1) * 512],
                        start=(kk == 0), stop=(kk == 3))
            t = ap_.tile([128, G3], BF16, tag=f"gx_{d}{mc}")
            for nb in range(3):
                nc.vector.tensor_tensor(
                    t[:, nb * 512:(nb + 1) * 512],
                    ps[:, nb * 512:(nb + 1) * 512],
                    gbias[d][:, nb * 512:(nb + 1) * 512], OP.add)
            gx[d].append(t)

    if stage == 2:   # + gi
        _dbg_out(gx["f"][0][0:1, 0:TB])
        return

    # ---------------- uncertainty features (tb-on-partition)
    NP = len(PAIRS)
    Nt = fp.tile([128, NCH, K], F32, tag="norms")
    Gt = fp.tile([128, NCH, NP], F32, tag="gram")
    St = fp.tile([128, NCH, K], F32, tag="sdot")
    Ns = fp.tile([128, 1], F32, tag="snorm")

    def ttr(in0, in1, acc):
        scr = sp.tile([128, E], BF16, tag="ttr_scr")
        nc.vector.scalar_tensor_tensor(
            out=scr[:], in0=in0, scalar=1.0, in1=in1,
            op0=OP.mult, op1=OP.mult, accum_out=acc)

    ttr(sent[:], sent[:], Ns[:])
    for c in range(NCH):
        for k in range(K):
            ttr(emb[k][:, c, :], emb[k][:, c, :], Nt[:, c, k:k + 1])
        for i, (k, j) in enumerate(PAIRS):
            ttr(emb[k][:, c, :], emb[j][:, c, :], Gt[:, c, i:i + 1])
        for k in range(K):
            ttr(emb[k][:, c, :], sent[:], St[:, c, k:k + 1])

    uf = fp.tile([128, NCH, 4 * K], BF16, tag="uf")  # feature row = k*4 + ci

    # min_dist: d2(k,j) = n_k + n_j - 2 g_kj; min over partners; sqrt
    npair = fp.tile([128, NCH, NP], F32, tag="npair")
    for i, (k, j) in enumerate(PAIRS):
        nc.vector.tensor_tensor(npair[:, :, i], Nt[:, :, k], Nt[:, :, j], OP.add)
    d2 = fp.tile([128, NCH, NP], F32, tag="d2")
    nc.vector.scalar_tensor_tensor(out=d2[:], in0=Gt[:], scalar=-2.0,
                                   in1=npair[:], op0=OP.mult, op1=OP.add)
    pidx = {}
    for i, (k, j) in enumerate(PAIRS):
        pidx[(k, j)] = i
        pidx[(j, k)] = i
    md = fp.tile([128, NCH, K], F32, tag="md")
    for k in range(K):
        parts = [pidx[(k, j)] for j in range(K) if j != k]
        nc.vector.tensor_tensor(md[:, :, k], d2[:, :, parts[0]],
                                d2[:, :, parts[1]], OP.min)
        for i in parts[2:]:
            nc.vector.tensor_tensor(md[:, :, k], md[:, :, k], d2[:, :, i], OP.min)
    nc.vector.tensor_scalar_max(md[:], md[:], 0.0)
    nc.scalar.activation(uf[:, :, 0::4], md[:], AF.Sqrt)

    # cos_dist (k>0): g_0k * rsqrt(n_0 * n_k); k=0 -> 0
    cprod = fp.tile([128, NCH, K - 1], F32, tag="cprod")
    for c in range(NCH):
        nc.vector.tensor_scalar(out=cprod[:, c, :], in0=Nt[:, c, 1:],
                                scalar1=Nt[:, c, 0:1], scalar2=None, op0=OP.mult)
    nc.vector.reciprocal(cprod[:], cprod[:])
    nc.scalar.activation(cprod[:], cprod[:], AF.Sqrt)
    nc.vector.tensor_tensor(uf[:, :, 5:24:4], Gt[:, :, 0:K - 1], cprod[:], OP.mult)
    nc.vector.memset(uf[:, :, 1], 0.0)

    # sentence_cos_dist: s_k * rsqrt(ns * n_k)
    sprod = fp.tile([128, NCH, K], F32, tag="sprod")
    nc.vector.tensor_scalar(out=sprod[:], in0=Nt[:], scalar1=Ns[:, 0:1],
                            scalar2=None, op0=OP.mult)
    nc.vector.reciprocal(sprod[:], sprod[:])
    nc.scalar.activation(sprod[:], sprod[:], AF.Sqrt)
    nc.vector.tensor_tensor(uf[:, :, 2::4], St[:], sprod[:], OP.mult)
    nc.vector.tensor_copy(uf[:, :, 3::4], probs[:])

    if stage == 3:   # + uncertainty features
        _dbg_out(uf[0:1, :, :].rearrange("p c k -> p (c k)"), n=NCH * 24)
        return

    # ---------------- uf -> [24, TB] via PE transpose, then the CNN
    ufT = fp.tile([24, TBP], BF16, tag="ufT")
    for c in range(NCH):
        tp = pc.tile([24, 128], BF16, tag="cnn_ps", name="uftp")
        nc.tensor.transpose(tp[:], uf[:, c, :], ident[:])
        nc.scalar.copy(ufT[:, c * 128:(c + 1) * 128], tp[:])
    win = []
    for l in range(4):
        t = fp.tile([12, TBP], BF16, tag=f"win{l}")
        nc.gpsimd.dma_start(t[:], ufT[4 * l:4 * l + 12, :])
        win.append(t)

    def mm_epilogue(ps, bias_tile, bias_col, tag, prelu=True, n=TB):
        t = fp.tile([128, n], BF16, tag=tag)
        if prelu:
            s = sp.tile([128, n], BF16, tag="epi_scr")
            nc.scalar.activation(s[:], ps, AF.Identity,
                                 bias=bias_tile[:, bias_col:bias_col + 1])
            nc.vector.scalar_tensor_tensor(out=t[:], in0=s[:], scalar=0.25,
                                           in1=s[:], op0=OP.mult, op1=OP.max)
        else:
            nc.scalar.activation(t[:], ps, AF.Identity,
                                 bias=bias_tile[:, bias_col:bias_col + 1])
        return t

    c1 = []
    for l in range(4):
        ps = pc.tile([128, TBP], F32, tag="cnn_ps")
        nc.tensor.matmul(ps[:], w1T[:], win[l][:], start=True, stop=True)
        c1.append(mm_epilogue(ps[:], b_c1, 0, f"c1_{l}", n=TBP))
    c2 = []
    for lp in range(2):
        for mc in range(2):
            ps = pc.tile([128, TBP], F32, tag="cnn_ps")
            for dk in range(3):
                nc.tensor.matmul(ps[:], w2T[dk][:, mc * 128:(mc + 1) * 128],
                                 c1[lp + dk][:], start=(dk == 0), stop=(dk == 2))
            c2.append(mm_epilogue(ps[:], b_c2, mc, f"c2_{lp}{mc}", n=TBP))
    unc = []
    for mc in range(4):
        ps = pc.tile([128, TBP], F32, tag="cnn_ps")
        for kk in range(4):
            nc.tensor.matmul(ps[:], w3T[kk][:, mc * 128:(mc + 1) * 128],
                             c2[kk][:], start=(kk == 0), stop=(kk == 3))
        unc.append(mm_epilogue(ps[:], b_c3, mc, f"unc{mc}", n=TBP))

    if stage == 4:   # + CNN
        _dbg_out(unc[0][0:1, 0:TB])
        return

    # ---------------- context features
    ps = pc.tile([128, TB], F32, tag="cnn_ps", name="pose_ps")
    nc.tensor.matmul(ps[:], posW[:], posT[:], start=True, stop=True)
    pose = fp.tile([128, TB], BF16, tag="pose")
    nc.scalar.copy(pose[:], ps[:])
    posf = []
    for mc in range(2):
        ps = pc.tile([128, TBP], F32, tag="cnn_ps")
        nc.tensor.matmul(ps[:, 0:TB], pWT[:, mc * 128:(mc + 1) * 128], pose[:],
                         start=True, stop=True)
        posf.append(mm_epilogue(ps[:, 0:TB], b_pb, mc, f"posf{mc}"))

    # img_feat: stream iWT chunks from DRAM, one Mc per pass (1 PSUM bank)
    imgf = []
    for mc in range(4):
        ips = pm.tile([128, TB], F32, tag="mm_ps0", name="mm_ps0")
        for kk in range(16):
            iwt = ws.tile([128, 512], BF16, tag="iw_s", name="iw_s")
            nc.sync.dma_start(iwt[:], io["iWT"][kk * 128:(kk + 1) * 128, :])
            nc.tensor.matmul(ips[:], iwt[:, mc * 128:(mc + 1) * 128],
                             att[kk][:], start=(kk == 0), stop=(kk == 15))
        imgf.append(mm_epilogue(ips[:], b_ib, mc, f"imgf{mc}"))

    hidf = []
    for mc in range(2):
        ps = pc.tile([128, TBP], F32, tag="cnn_ps")
        for kk in range(4):
            nc.tensor.matmul(ps[:, 0:TB], hWT[kk][:, mc * 128:(mc + 1) * 128],
                             hid[kk][:], start=(kk == 0), stop=(kk == 3))
        hidf.append(mm_epilogue(ps[:, 0:TB], b_hb, mc, f"hidf{mc}"))

    if stage == 5:   # + pos/img/hid feats
        _dbg_out(imgf[0][0:1, 0:TB])
        return

    # ---------------- GRU (both directions, batch 16 each)
    h = ap_.tile([48, H], BF16, tag="h")           # rows 0:16 fwd, 32:48 bwd
    nc.vector.memset(h[:], 0.0)
    hT = {d: ap_.tile([128, 4, BPC], BF16, tag=f"hT_{d}", name=f"hT_{d}") for d in "fb"}
    for d in "fb":
        nc.vector.memset(hT[d][:], 0.0)
    rowof = {"f": 0, "b": 32}

    for t_ in range(T):
        ghp = pg.tile([128, G3], F32, tag="gh_ps", name="ghp")
        for d in "fb":
            r0 = rowof[d]
            for kk in range(4):
                for nb in range(3):
                    nc.tensor.matmul(
                        ghp[r0:r0 + 16, nb * 512:(nb + 1) * 512],
                        hT[d][:, kk, :],
                        whh[d][kk][:, nb * 512:(nb + 1) * 512],
                        start=(kk == 0), stop=(kk == 3))
        rzp = sp.tile([48, 2 * H], BF16, tag="rzp")
        npre = sp.tile([48, H], BF16, tag="npre")
        c, off = (16 * t_) // 128, (16 * t_) % 128
        gxs = sp.tile([48, G3], BF16, tag="gxs", bufs=4, name="gxs")
        nc.gpsimd.dma_start(gxs[0:16, :], gx["f"][c][off:off + 16, :])
        nc.gpsimd.dma_start(gxs[32:48, :], gx["b"][c][off:off + 16, :])
        rz = sp.tile([48, 2 * H], BF16, tag="rz")
        # r-gate path first: it gates n and is on the critical chain
        nc.vector.tensor_tensor(rzp[:, 0:H], ghp[0:48, 0:H], gxs[:, 0:H], OP.add)
        nc.scalar.activation(rz[:, 0:H], rzp[:, 0:H], AF.Sigmoid)
        nc.vector.tensor_tensor(npre[:], ghp[0:48, 2 * H:], rz[:, 0:H], OP.mult)
        nc.vector.tensor_tensor(npre[:], npre[:], gxs[:, 2 * H:], OP.add)
        n_ = sp.tile([48, H], BF16, tag="n_")
        nc.scalar.activation(n_[:], npre[:], AF.Tanh)
        # z-gate off the critical chain
        nc.vector.tensor_tensor(rzp[:, H:2 * H], ghp[0:48, H:2 * H],
                                gxs[:, H:2 * H], OP.add)
        nc.scalar.activation(rz[:, H:2 * H], rzp[:, H:2 * H], AF.Sigmoid)
        # h-update chunked by 128 cols so each hT transpose (and the next
        # step's first K-chunk matmul) fires as soon as its slice is ready
        hmn = sp.tile([48, H], BF16, tag="hmn")
        for cc in range(4):
            cs = slice(cc * 128, (cc + 1) * 128)
            nc.vector.tensor_tensor(hmn[:, cs], h[:, cs], n_[:, cs], OP.subtract)
            nc.vector.tensor_tensor(hmn[:, cs], hmn[:, cs],
                                    rz[:, H + cc * 128:H + (cc + 1) * 128], OP.mult)
            nc.vector.tensor_tensor(h[:, cs], n_[:, cs], hmn[:, cs], OP.add)
            if t_ < T - 1:
                for d in "fb":
                    r0 = rowof[d]
                    nc.sync.dma_start_transpose(
                        hT[d][:, cc, :], h[r0:r0 + 16, cs])

    if stage == 6:   # + GRU
        _dbg_out(h[0:1, 0:TB])
        return

    # final hidden -> cap_feat rhs chunks (broadcast across t)
    capb = []
    for d in "fb":
        r0 = rowof[d]
        for cc in range(4):
            hc = ap_.tile([128, BPC], BF16, tag=f"hlast_{d}{cc}")
            nc.sync.dma_start_transpose(hc[:], h[r0:r0 + 16, cc * 128:(cc + 1) * 128])
            t = fp.tile([128, TB], BF16, tag=f"capb_{d}{cc}")
            nc.sync.dma_start(t[:].rearrange("p (t b) -> p t b", t=T),
                              hc[:].unsqueeze(1).broadcast_to([128, T, BPC]))
            capb.append(t)

    # ---------------- ctx = prelu(concat @ cW.T + cb), then logits
    rhs_ctx = capb + posf + imgf + hidf            # 8+2+4+2 = 16 chunks
    ctxa = []
    for mc in range(4):
        cps = pm.tile([128, TB], F32, tag="mm_ps0", name="mm_ps0")
        for kk in range(16):
            cwt = ws.tile([128, 512], BF16, tag="cw_s", name="cw_s")
            nc.sync.dma_start(cwt[:], io["cWT"][kk * 128:(kk + 1) * 128, :])
            nc.tensor.matmul(cps[:], cwt[:, mc * 128:(mc + 1) * 128],
                             rhs_ctx[kk][:], start=(kk == 0), stop=(kk == 15))
        ctxa.append(mm_epilogue(cps[:], b_cb, mc, f"ctxa{mc}"))

    psl = pc.tile([1, TB], F32, tag="cnn_ps", name="lg_ps")
    rhs_o = [t[:] for t in ctxa] + [t[:, 0:TB] for t in unc]
    for kk in range(8):
        nc.tensor.matmul(psl[:], oWT[kk][:], rhs_o[kk],
                         start=(kk == 0), stop=(kk == 7))
    lg = fp.tile([1, TB], F32, tag="lg")
    nc.scalar.copy(lg[:], psl[:])
    nc.sync.dma_start(io["out_logits"][:], lg[:])
    if close:
        ctx.close()


# ---------------------------------------------------------------- build

@functools.lru_cache(maxsize=4)
def _build(stage=99):
    nc = bacc.Bacc("TRN2", target_bir_lowering=False, debug=False,
                   enable_asserts=False, num_devices=NCORE)
    io = {}

    def din(name, shape, dt):
        io[name] = nc.dram_tensor(name, list(shape), dt, kind="ExternalInput").ap()

    din("attT", [IC, TB], BF16)
    din("hidT", [512, TB], BF16)
    din("posT", [51, TB], BF16)
    din("probsP", [TBP, K], F32)
    din("sentB", [128, E], BF16)
    din("ident", [128, 128], BF16)
    din("cap_emb_bf", [CV, E], BF16)
    din("cap_emb_w_bf", [CV, E], BF16)
    din("gidx", [128, 24], mybir.dt.int32)
    din("iWT", [IC, 512], BF16)
    din("cWT", [IC, 512], BF16)
    din("hWT", [512, 256], BF16)
    din("pWT", [128, 256], BF16)
    din("posW", [51, 128], BF16)
    din("w1T", [12, 128], BF16)
    din("w2T", [384, 256], BF16)
    din("w3T", [512, 512], BF16)
    din("oWT", [1024, 1], BF16)
    for d in "fb":
        din(f"wihT_{d}", [512, G3], BF16)
        din(f"whhT_{d}", [512, G3], BF16)
        din(f"gbias_{d}", [128, G3], BF16)
    for nm, sz in (("ib", 512), ("pb", 256), ("hb", 256), ("cb", 512),
                   ("b1", 128), ("b2", 256), ("b3", 512)):
        din(nm, [sz], F32)
    io["out_logits"] = nc.dram_tensor("out_logits", [1, TB], F32,
                                      kind="ExternalOutput").ap()

    with tile.TileContext(nc) as tc:
        _emit(nc, tc, io, stage)
    nc.compile()
    return nc


# ---------------------------------------------------------------- host side

def _bf(x):
    return np.ascontiguousarray(np.asarray(x, np.float32).astype(BF))


def _wrap_idx(idx, n=TBP):
    """Flat index list -> [16, n/16] int16 wrapped layout (i -> [i%16, i//16])."""
    a = np.full(n, -1, np.int64)
    a[:len(idx)] = idx
    return np.ascontiguousarray(a.reshape(n // 16, 16).T.astype(np.int16))


def _prep_core(ci, inp, shared):
    sl = slice(ci * BPC, (ci + 1) * BPC)
    attT = _bf(np.asarray(inp["attended_img"])[sl].transpose(2, 1, 0).reshape(IC, TB))
    hidT = _bf(np.asarray(inp["hidden"])[sl].transpose(2, 1, 0).reshape(512, TB))
    posT = _bf(np.asarray(inp["pos"])[sl].transpose(2, 1, 0).reshape(51, TB))
    probsP = np.zeros((TBP, K), np.float32)
    probsP[:TB] = np.asarray(inp["topk_probs"])[:, sl, :].reshape(TB, K)
    cap = np.asarray(inp["caption"])[sl].astype(np.int64)     # [16, 21]
    tw = np.asarray(inp["topk_words"])[:, sl, :].astype(np.int64)  # [21, 16, 6]
    def pad384(a):
        o = np.zeros(TBP, np.int64)
        o[:TB] = a
        return o

    gidx = np.zeros((128, 24), np.int32)
    for k in range(K):
        col = pad384(tw[:, :, k].reshape(TB))
        for c in range(NCH):
            gidx[:, k * NCH + c] = col[c * 128:(c + 1) * 128]
    for di, order in enumerate((cap.T.reshape(TB), cap[:, ::-1].T.reshape(TB))):
        col = pad384(order)
        for c in range(NCH):
            gidx[:, 18 + di * NCH + c] = col[c * 128:(c + 1) * 128]
    m = {
        "attT": attT, "hidT": hidT, "posT": posT, "probsP": probsP,
        "sentB": np.ascontiguousarray(np.tile(shared["sent"][sl], (8, 1))),
        "gidx": gidx,
    }
    m.update(shared["weights"])
    return m


def _prep_shared(inp):
    cap_emb = np.asarray(inp["cap_embedding"], np.float32)
    capt = np.asarray(inp["caption"]).astype(np.int64)
    cap_len = np.asarray(inp["cap_len"]).astype(np.int64)
    mask = (np.arange(T)[None, :] < cap_len[:, None]).astype(np.float32)
    sent = np.einsum("bte,bt->be", cap_emb[capt], mask)       # [B, E]

    w = {}
    w["cap_emb_bf"] = _bf(cap_emb)
    w["cap_emb_w_bf"] = _bf(inp["cap_emb_w"])
    w["iWT"] = _bf(np.asarray(inp["iW"], np.float32).T)
    w["cWT"] = _bf(np.asarray(inp["cW"], np.float32).T)
    w["hWT"] = _bf(np.asarray(inp["hW"], np.float32).T)
    w["pWT"] = _bf(np.asarray(inp["pW"], np.float32).T)
    w["posW"] = _bf(inp["pos_emb_w"])
    w["w1T"] = _bf(np.asarray(inp["conv1_w"], np.float32).transpose(2, 1, 0).reshape(12, 128))
    w["w2T"] = _bf(np.asarray(inp["conv2_w"], np.float32).transpose(2, 1, 0).reshape(384, 256))
    w["w3T"] = _bf(np.asarray(inp["conv3_w"], np.float32).transpose(2, 1, 0).reshape(512, 512))
    w["oWT"] = _bf(np.asarray(inp["oW"], np.float32).T)
    for d, sfx in (("f", "_f"), ("b", "_b")):
        w[f"wihT_{d}"] = _bf(np.asarray(inp["gru_w_ih" + sfx], np.float32).T)
        w[f"whhT_{d}"] = _bf(np.asarray(inp["gru_w_hh" + sfx], np.float32).T)
        gb = (np.asarray(inp["gru_b_ih" + sfx], np.float32)
              + np.asarray(inp["gru_b_hh" + sfx], np.float32)).reshape(1, G3)
        w[f"gbias_{d}"] = _bf(np.tile(gb, (128, 1)))
    for nm, key in (("ib", "ib"), ("pb", "pb"), ("hb", "hb"), ("cb", "cb"),
                    ("b1", "conv1_b"), ("b2", "conv2_b"), ("b3", "conv3_b")):
        w[nm] = np.ascontiguousarray(np.asarray(inp[key], np.float32))
    w["ident"] = _bf(np.eye(128, dtype=np.float32))
    return {"weights": w, "sent": _bf(sent).astype(BF)}


_RUNNER = {}


def _get_runner():
    if "fn" not in _RUNNER:
        nc = _build()
        _RUNNER["nc"] = nc
        _RUNNER["fn"] = lambda in_maps: run_bass_kernel_spmd(
            nc, in_maps, core_ids=list(range(NCORE)))
    return _RUNNER["fn"]


def kernel(**inputs):
    fn = _get_runner()
    shared = _prep_shared(inputs)
    in_maps = [_prep_core(ci, inputs, shared) for ci in range(NCORE)]
    res = fn(in_maps)
    logits = np.zeros((B, T), np.float32)
    for ci in range(NCORE):
        lg = np.asarray(res.results[ci]["out_logits"], np.float32).reshape(TB)
        logits[ci * BPC:(ci + 1) * BPC] = lg.reshape(T, BPC).T
    logits += float(np.asarray(inputs["ob"]).reshape(-1)[0])
    pos = np.asarray(inputs["pos"])
    valid_pos = np.argmax(pos, axis=2) != (pos.shape[-1] - 2)
    return logits, valid_pos



# revision 5
# speedup vs baseline: 4.5823x; 4.5823x over previous
"""Trainium2 Bass kernel for nn_DecisionMaker (retrieval_knn).

Strategy (v2): pure data-parallel SPMD over batch. B=128 is split into 8
slices of 16; every NeuronCore runs the identical program on its slice.
Weights are replicated (host pre-transposes them into the layouts the
tensor engine wants); embedding-table rows are fetched with dma_gather.

Per-core layout: tb = t*16 + b_local (t-major), tb in [0, 336). Feature
activations are "feature-on-partition": [F (128-chunks), tb (free)].

GRU (v2): gates-on-partition. Per direction, hidden state lives as
hT [128 (hid chunk cc), 4, 16 (batch)]. The recurrent matmul makes the
weight chunk the stationary operand and hT the 16-col moving operand:
48 tiny matmuls/step/dir accumulating into one PSUM bank shaped
[128 (gate), 12 (gate chunk), 16 (batch)]. Gate math operates directly
on that layout, the h-update writes hT in place, so the loop needs no
transposes and no DMA. gx comes precomputed in the same layout
(gxT [128, 12, tb]) so each step's input-gate term is a free AP slice;
the backward direction reads token slice (T-1-t) of the forward-order
gxT, which also kills the reversed caption gather of v1.
"""

import functools

import numpy as np
import ml_dtypes

import concourse.bass as bass
import concourse.tile as tile
from concourse import bacc, mybir
from concourse.bass_utils import run_bass_kernel_spmd

F32 = mybir.dt.float32
BF16 = mybir.dt.bfloat16
AF = mybir.ActivationFunctionType
OP = mybir.AluOpType
BF = ml_dtypes.bfloat16

NCORE = 8
B, T, K, E, H, IC = 128, 21, 6, 512, 512, 2048
CV = 10001
BPC = B // NCORE            # 16 batch rows per core
TB = T * BPC                # 336 (t-major: tb = t*16 + b)
TBP = 384                   # padded to 3*128 for gathers
NCH = 3                     # tb chunks of 128
G3 = 3 * H                  # 1536 gate width
GC = G3 // 128              # 12 gate chunks
PAIRS = [(i, j) for i in range(K) for j in range(i + 1, K)]  # 15, (0,k) first


def _emit(nc, tc, io, stage=99):
    from contextlib import ExitStack
    ctx = ExitStack()
    wp = ctx.enter_context(tc.tile_pool(name="weights", bufs=1))
    ws = ctx.enter_context(tc.tile_pool(name="wstream", bufs=3))
    ap_ = ctx.enter_context(tc.tile_pool(name="acts", bufs=1))
    gp = ctx.enter_context(tc.tile_pool(name="gather", bufs=1))
    sp = ctx.enter_context(tc.tile_pool(name="scratch", bufs=2))
    fp = ctx.enter_context(tc.tile_pool(name="feats", bufs=1))
    pa = ctx.enter_context(tc.tile_pool(name="psum_mm", bufs=4, space="PSUM"))
    pg = ctx.enter_context(tc.tile_pool(name="psum_gru", bufs=2, space="PSUM"))
    nrep = 8 if stage == 98 else 1
    for _rep in range(nrep):
        _emit_body(nc, tc, io, 99 if stage == 98 else stage, ctx,
                   wp, ws, ap_, gp, sp, fp, pa, pg, close=(_rep == nrep - 1))


def _emit_body(nc, tc, io, stage, ctx, wp, ws, ap_, gp, sp, fp, pa, pg,
               close=True):

    # ---------------- gathers first on the gpsimd queue (they gate the
    # uncertainty dots and gi); bulk weight loads go on the sync queue.
    gidx = gp.tile([128, 21], mybir.dt.int32, tag="gidx")
    nc.gpsimd.dma_start(gidx[:], io["gidx"][:])

    def gather_rows(dst, table, col):
        nc.gpsimd.indirect_dma_start(
            out=dst, out_offset=None, in_=table,
            in_offset=bass.IndirectOffsetOnAxis(ap=gidx[:, col:col + 1], axis=0))

    emb = []
    for k in range(K):
        out = gp.tile([128, NCH, E], BF16, tag=f"emb{k}", name=f"emb{k}")
        for c in range(NCH):
            gather_rows(out[:, c, :], io["cap_emb_bf"][:], k * NCH + c)
        emb.append(out)

    capeN = gp.tile([128, NCH, E], BF16, tag="capeN", name="capeN")
    for c in range(NCH):
        gather_rows(capeN[:, c, :], io["cap_emb_w_bf"][:], 18 + c)
    # capeT: [128 (E chunk), 4, tok] via xbar transposes on the ACT queue
    capeT = gp.tile([128, 4, TBP], BF16, tag="capeT", name="capeT")
    for kk in range(4):
        for c in range(NCH):
            nc.scalar.dma_start_transpose(
                capeT[:, kk, c * 128:(c + 1) * 128],
                capeN[:, c, kk * 128:(kk + 1) * 128])

    # ---------------- resident weights / activations (sync queue)
    def load_w(name, shape, dt=BF16, src=None):
        t = wp.tile(list(shape), dt, tag=name)
        nc.sync.dma_start(t[:], io[name][:] if src is None else src)
        return t

    def load_w_chunks(name, rows, cols, dt=BF16):
        ts = []
        for k in range(rows // 128):
            t = wp.tile([128, cols], dt, tag=f"{name}{k}")
            nc.sync.dma_start(t[:], io[name][k * 128:(k + 1) * 128, :])
            ts.append(t)
        return ts

    def load_bias(name, f):
        p = min(f, 128)
        m = max(1, f // 128)
        t = wp.tile([p, m], F32, tag=name)
        src = io[name].rearrange("(m p) -> p m", p=p) if f > 128 \
            else io[name].unsqueeze(1)
        nc.sync.dma_start(t[:], src)
        return t

    sent = ap_.tile([128, E], BF16, tag="sent")   # pre-broadcast rows p -> b=p%16
    nc.sync.dma_start(sent[:], io["sentB"][:])
    probs = ap_.tile([128, NCH, K], F32, tag="probs")
    nc.sync.dma_start(probs[:], io["probsP"].rearrange("(c p) k -> p c k", p=128))
    posT = ap_.tile([51, TB], BF16, tag="posT")
    nc.sync.dma_start(posT[:], io["posT"][:])
    gbias = {d: load_w(f"gbias_{d}", [128, GC], F32) for d in "fb"}
    b_ib = load_bias("ib", 512)
    b_pb = load_bias("pb", 256)
    b_hb = load_bias("hb", 256)
    b_cb = load_bias("cb", 512)
    b_c1 = load_bias("b1", 128)
    b_c2 = load_bias("b2", 256)
    b_c3 = load_bias("b3", 512)
    ident = load_w("ident", [128, 128])           # bf16 identity (PE transpose)
    w1T = load_w("w1T", [12, 128])
    w2T = load_w_chunks("w2T", 384, 256)          # 3 x [128, 256]
    w3T = load_w_chunks("w3T", 512, 512)          # 4 x [128, 512]
    oWT = load_w("oWT", [128, 8])
    pWT = load_w("pWT", [128, 256])
    posW = load_w("posW", [51, 128])              # pos_emb_w, natural lhsT
    hWT = wp.tile([128, 4, 256], BF16, tag="hWT")
    nc.sync.dma_start(hWT[:], io["hWT"].rearrange("(k p) c -> p k c", p=128))
    wih = {d: load_w_chunks(f"wihT_{d}", 512, G3) for d in "fb"}
    whh = {d: load_w_chunks(f"whhT_{d}", 512, G3) for d in "fb"}
    att = []
    for k in range(IC // 128):
        t = ap_.tile([128, TB], BF16, tag=f"att{k}")
        nc.sync.dma_start(t[:], io["attT"][k * 128:(k + 1) * 128, :])
        att.append(t)
    hid = []
    for k in range(4):
        t = ap_.tile([128, TB], BF16, tag=f"hid{k}")
        nc.sync.dma_start(t[:], io["hidT"][k * 128:(k + 1) * 128, :])
        hid.append(t)

    def _dbg_out(src, n=TB):
        lgd = fp.tile([1, TB], F32, tag="lg", name="lgdbg")
        nc.vector.memset(lgd[:], 0.0)
        nc.vector.tensor_copy(lgd[0:1, 0:n], src)
        nc.sync.dma_start(io["out_logits"][:], lgd[:])
        ctx.close()

    if stage == 0:   # loads only
        _dbg_out(att[0][0:1, 0:TB])
        return
    if stage == 1:   # gathers
        _dbg_out(emb[0][0:1, 0, 0:TB])
        return

    # ---------------- uncertainty dot products, split across engines.
    # HW limits: accum ops exist only on DVE (stt+accum) and ACT
    # (Square+accum); gpsimd has neither, but can compute pair diffs.
    NP = len(PAIRS)
    NDIR = 4                        # tail pairs routed as direct |ek-ej|^2
    NGRAM = NP - NDIR               # leading pairs via Gram dots on DVE
    Nt = fp.tile([128, NCH, K], F32, tag="norms")
    Gt = fp.tile([128, NCH, NGRAM], F32, tag="gram")
    St = fp.tile([128, NCH, K], F32, tag="sdot")
    Ns = fp.tile([128, 1], F32, tag="snorm")
    d2 = fp.tile([128, NCH, NP], F32, tag="d2")

    def sq_acc(a, acc):             # ACT: acc = sum(a*a) over free dim
        scr = sp.tile([128, E], BF16, tag="sq_scr", bufs=3, name="sq_scr")
        nc.scalar.activation(scr[:], a, AF.Square, accum_out=acc)

    def dot_dve(a, b, acc):         # DVE: acc = sum(a*b)
        scr = sp.tile([128, E], BF16, tag="d_scr", bufs=3, name="d_scr")
        nc.vector.scalar_tensor_tensor(
            out=scr[:], in0=a, scalar=1.0, in1=b,
            op0=OP.mult, op1=OP.mult, accum_out=acc)

    sq_acc(sent[:], Ns[:])
    for c in range(NCH):
        for k in range(K):          # norms on ACT
            sq_acc(emb[k][:, c, :], Nt[:, c, k:k + 1])
        for i, (k, j) in enumerate(PAIRS):
            if i < NGRAM:           # Gram dots on DVE
                dot_dve(emb[k][:, c, :], emb[j][:, c, :], Gt[:, c, i:i + 1])
            else:                   # d2 direct: gpsimd diff + ACT square
                df = sp.tile([128, E], BF16, tag="df_scr", bufs=3, name="df_scr")
                nc.gpsimd.tensor_tensor(df[:], emb[k][:, c, :],
                                        emb[j][:, c, :], OP.subtract)
                sq_acc(df[:], d2[:, c, i:i + 1])
        for k in range(K):          # sentence dots on DVE
            dot_dve(emb[k][:, c, :], sent[:], St[:, c, k:k + 1])

    # ---------------- gi -> gxT[d] [128 (gate), 12, tok] (+ bias)
    gxT = {}
    for di, d in enumerate("fb"):
        gxT[d] = ap_.tile([128, GC, TB], BF16, tag=f"gxT_{d}", name=f"gxT_{d}")
        for gc in range(GC):
            ps = pa.tile([128, TBP], F32, tag="mm", name="gi_ps")
            for kk in range(4):
                nc.tensor.matmul(
                    ps[:, 0:TB], wih[d][kk][:, gc * 128:(gc + 1) * 128],
                    capeT[:, kk, 0:TB], start=(kk == 0), stop=(kk == 3))
            if (gc + di) % 2 == 0:
                nc.vector.tensor_scalar(out=gxT[d][:, gc, :], in0=ps[:, 0:TB],
                                        scalar1=gbias[d][:, gc:gc + 1],
                                        scalar2=None, op0=OP.add)
            else:
                nc.scalar.activation(gxT[d][:, gc, :], ps[:, 0:TB], AF.Identity,
                                     bias=gbias[d][:, gc:gc + 1])

    if stage == 2:   # + gi
        _dbg_out(gxT["f"][0:1, 0, 0:TB])
        return

    # ---------------- uncertainty features -> uf [128, c, k*4+ci]
    uf = fp.tile([128, NCH, 4 * K], BF16, tag="uf")

    # min_dist: gram pairs d2 = n_k + n_j - 2 g_kj (tail pairs came direct);
    # min over partners; sqrt on ACT (Sqrt shares the pre-GRU table set)
    npair = fp.tile([128, NCH, NGRAM], F32, tag="npair")
    for i, (k, j) in enumerate(PAIRS[:NGRAM]):
        nc.vector.tensor_tensor(npair[:, :, i], Nt[:, :, k], Nt[:, :, j], OP.add)
    nc.vector.scalar_tensor_tensor(out=d2[:, :, 0:NGRAM], in0=Gt[:],
                                   scalar=-2.0, in1=npair[:],
                                   op0=OP.mult, op1=OP.add)
    pidx = {}
    for i, (k, j) in enumerate(PAIRS):
        pidx[(k, j)] = i
        pidx[(j, k)] = i
    md = fp.tile([128, NCH, K], F32, tag="md")
    for k in range(K):
        parts = [pidx[(k, j)] for j in range(K) if j != k]
        nc.vector.tensor_tensor(md[:, :, k], d2[:, :, parts[0]],
                                d2[:, :, parts[1]], OP.min)
        for i in parts[2:]:
            nc.vector.tensor_tensor(md[:, :, k], md[:, :, k], d2[:, :, i], OP.min)
    nc.vector.tensor_scalar_max(md[:], md[:], 0.0)
    nc.scalar.activation(uf[:, :, 0::4], md[:], AF.Sqrt)

    # cos_dist (k>0): g_0k * rsqrt(n_0 * n_k); k=0 -> 0
    cprod = fp.tile([128, NCH, K - 1], F32, tag="cprod")
    for c in range(NCH):
        nc.vector.tensor_scalar(out=cprod[:, c, :], in0=Nt[:, c, 1:],
                                scalar1=Nt[:, c, 0:1], scalar2=None, op0=OP.mult)
    nc.vector.reciprocal(cprod[:], cprod[:])
    nc.scalar.activation(cprod[:], cprod[:], AF.Sqrt)
    nc.vector.tensor_tensor(uf[:, :, 5:24:4], Gt[:, :, 0:K - 1], cprod[:],
                            OP.mult)
    nc.vector.memset(uf[:, :, 1], 0.0)

    # sentence_cos_dist: s_k * rsqrt(ns * n_k)
    sprod = fp.tile([128, NCH, K], F32, tag="sprod")
    nc.vector.tensor_scalar(out=sprod[:], in0=Nt[:], scalar1=Ns[:, 0:1],
                            scalar2=None, op0=OP.mult)
    nc.vector.reciprocal(sprod[:], sprod[:])
    nc.scalar.activation(sprod[:], sprod[:], AF.Sqrt)
    nc.vector.tensor_tensor(uf[:, :, 2::4], St[:], sprod[:], OP.mult)
    nc.vector.tensor_copy(uf[:, :, 3::4], probs[:])

    if stage == 3:   # + uncertainty features
        _dbg_out(uf[0:1, :, :].rearrange("p c k -> p (c k)"), n=NCH * 24)
        return

    # ---------------- uf -> [24, TB] via PE transpose, then the CNN
    ufT = fp.tile([24, TBP], BF16, tag="ufT")
    for c in range(NCH):
        tp = pa.tile([24, 128], BF16, tag="mm", name="uftp")
        nc.tensor.transpose(tp[:], uf[:, c, :], ident[:])
        nc.scalar.copy(ufT[:, c * 128:(c + 1) * 128], tp[:])
    win = []
    for l in range(4):
        t = fp.tile([12, TBP], BF16, tag=f"win{l}")
        nc.gpsimd.dma_start(t[:], ufT[4 * l:4 * l + 12, :])
        win.append(t)

    def mm_epilogue(ps, bias_tile, bias_col, tag, prelu=True, n=TB):
        t = fp.tile([128, n], BF16, tag=tag)
        if prelu:
            s = sp.tile([128, n], BF16, tag="epi_scr", bufs=3, name="epi_scr")
            nc.scalar.activation(s[:], ps, AF.Identity,
                                 bias=bias_tile[:, bias_col:bias_col + 1])
            nc.vector.scalar_tensor_tensor(out=t[:], in0=s[:], scalar=0.25,
                                           in1=s[:], op0=OP.mult, op1=OP.max)
        else:
            nc.scalar.activation(t[:], ps, AF.Identity,
                                 bias=bias_tile[:, bias_col:bias_col + 1])
        return t

    c1 = []
    for l in range(4):
        ps = pa.tile([128, TBP], F32, tag="mm", name="c1_ps")
        nc.tensor.matmul(ps[:], w1T[:], win[l][:], start=True, stop=True)
        c1.append(mm_epilogue(ps[:], b_c1, 0, f"c1_{l}", n=TBP))
    c2 = []
    for lp in range(2):
        for mc in range(2):
            ps = pa.tile([128, TBP], F32, tag="mm", name="c2_ps")
            for dk in range(3):
                nc.tensor.matmul(ps[:], w2T[dk][:, mc * 128:(mc + 1) * 128],
                                 c1[lp + dk][:], start=(dk == 0), stop=(dk == 2))
            c2.append(mm_epilogue(ps[:], b_c2, mc, f"c2_{lp}{mc}", n=TBP))
    unc = []
    for mc in range(4):
        ps = pa.tile([128, TBP], F32, tag="mm", name="c3_ps")
        for kk in range(4):
            nc.tensor.matmul(ps[:], w3T[kk][:, mc * 128:(mc + 1) * 128],
                             c2[kk][:], start=(kk == 0), stop=(kk == 3))
        unc.append(mm_epilogue(ps[:], b_c3, mc, f"unc{mc}", n=TBP))

    if stage == 4:   # + CNN
        _dbg_out(unc[0][0:1, 0:TB])
        return

    # ---------------- context features
    ps = pa.tile([128, TBP], F32, tag="mm", name="pose_ps")
    nc.tensor.matmul(ps[:, 0:TB], posW[:], posT[:], start=True, stop=True)
    pose = fp.tile([128, TB], BF16, tag="pose")
    nc.scalar.copy(pose[:], ps[:, 0:TB])
    posf = []
    for mc in range(2):
        ps = pa.tile([128, TBP], F32, tag="mm", name="posf_ps")
        nc.tensor.matmul(ps[:, 0:TB], pWT[:, mc * 128:(mc + 1) * 128], pose[:],
                         start=True, stop=True)
        posf.append(mm_epilogue(ps[:, 0:TB], b_pb, mc, f"posf{mc}"))

    # img_feat: stream iWT once (kk outer), 4 live PSUM banks
    ips = [pa.tile([128, TBP], F32, tag="mm", name=f"ips{mc}") for mc in range(4)]
    for kk in range(16):
        iwt = ws.tile([128, 512], BF16, tag="iw_s", name="iw_s")
        nc.sync.dma_start(iwt[:], io["iWT"][kk * 128:(kk + 1) * 128, :])
        for mc in range(4):
            nc.tensor.matmul(ips[mc][:, 0:TB], iwt[:, mc * 128:(mc + 1) * 128],
                             att[kk][:], start=(kk == 0), stop=(kk == 15))
    imgf = [mm_epilogue(ips[mc][:, 0:TB], b_ib, mc, f"imgf{mc}")
            for mc in range(4)]

    hidf = []
    for mc in range(2):
        ps = pa.tile([128, TBP], F32, tag="mm", name="hidf_ps")
        for kk in range(4):
            nc.tensor.matmul(ps[:, 0:TB], hWT[:, kk, mc * 128:(mc + 1) * 128],
                             hid[kk][:], start=(kk == 0), stop=(kk == 3))
        hidf.append(mm_epilogue(ps[:, 0:TB], b_hb, mc, f"hidf{mc}"))

    if stage == 5:   # + pos/img/hid feats
        _dbg_out(imgf[0][0:1, 0:TB])
        return

    # cWT loads queue on SP here; transfers run while the GRU computes
    cWT = load_w_chunks("cWT", IC, 512)

    # ---------------- GRU (both directions, gates-on-partition)
    hT = {}
    for d in "fb":
        hT[d] = ap_.tile([128, 4, BPC], BF16, tag=f"hT_{d}", name=f"hT_{d}")
        nc.vector.memset(hT[d][:], 0.0)

    for t_ in range(T):
        for d in "fb":
            ts = BPC * t_ if d == "f" else BPC * (T - 1 - t_)
            ghp = pg.tile([128, GC, BPC], F32, tag=f"ghp_{d}", name=f"ghp_{d}")
            for gc in range(GC):      # r(0:4), z(4:8), n(8:12)
                for kk in range(4):
                    nc.tensor.matmul(
                        ghp[:, gc, :],
                        whh[d][kk][:, gc * 128:(gc + 1) * 128],
                        hT[d][:, kk, :], start=(kk == 0), stop=(kk == 3))
            rz = sp.tile([128, 8, BPC], BF16, tag=f"rz_{d}", bufs=2, name="rz")
            nc.vector.tensor_tensor(rz[:], ghp[:, 0:8, :],
                                    gxT[d][:, 0:8, ts:ts + BPC], OP.add)
            sg = sp.tile([128, 8, BPC], BF16, tag=f"sg_{d}", bufs=2, name="sg")
            nc.scalar.activation(sg[:], rz[:], AF.Sigmoid)
            npre = sp.tile([128, 4, BPC], BF16, tag=f"np_{d}", bufs=2, name="npre")
            nc.vector.tensor_tensor(npre[:], ghp[:, 8:12, :], sg[:, 0:4, :],
                                    OP.mult)
            nc.vector.tensor_tensor(npre[:], npre[:],
                                    gxT[d][:, 8:12, ts:ts + BPC], OP.add)
            n_ = sp.tile([128, 4, BPC], BF16, tag=f"n_{d}", bufs=2, name="n_")
            nc.scalar.activation(n_[:], npre[:], AF.Tanh)
            # h = n + z*(h - n), on gpsimd (idle during the loop)
            hmn = sp.tile([128, 4, BPC], BF16, tag=f"hm_{d}", bufs=2, name="hmn")
            nc.gpsimd.tensor_tensor(hmn[:], hT[d][:], n_[:], OP.subtract)
            nc.gpsimd.tensor_tensor(hmn[:], hmn[:], sg[:, 4:8, :], OP.mult)
            nc.gpsimd.tensor_tensor(hT[d][:], n_[:], hmn[:], OP.add)

    if stage == 6:   # + GRU
        _dbg_out(hT["f"][0:1, :, :].rearrange("p a b -> p (a b)"), n=64)
        return

    # final hidden -> cap_feat rhs chunks (broadcast across t)
    capb = []
    for d in "fb":
        for cc in range(4):
            t = fp.tile([128, TB], BF16, tag=f"capb_{d}{cc}")
            nc.sync.dma_start(t[:].rearrange("p (t b) -> p t b", t=T),
                              hT[d][:, cc, :].unsqueeze(1)
                              .broadcast_to([128, T, BPC]))
            capb.append(t)

    # ---------------- ctx = prelu(concat @ cW.T + cb), then logits
    rhs_ctx = capb + posf + imgf + hidf            # 8+2+4+2 = 16 chunks
    cps = [pa.tile([128, TBP], F32, tag="mm", name=f"cps{mc}") for mc in range(4)]
    for kk in range(16):
        for mc in range(4):
            nc.tensor.matmul(cps[mc][:, 0:TB], cWT[kk][:, mc * 128:(mc + 1) * 128],
                             rhs_ctx[kk][:], start=(kk == 0), stop=(kk == 15))
    ctxa = [mm_epilogue(cps[mc][:, 0:TB], b_cb, mc, f"ctxa{mc}")
            for mc in range(4)]

    psl = pa.tile([1, TB], F32, tag="mm", name="lg_ps")
    rhs_o = [t[:] for t in ctxa] + [t[:, 0:TB] for t in unc]
    for kk in range(8):
        nc.tensor.matmul(psl[:], oWT[:, kk:kk + 1], rhs_o[kk],
                         start=(kk == 0), stop=(kk == 7))
    lg = fp.tile([1, TB], F32, tag="lg")
    nc.scalar.copy(lg[:], psl[:])
    nc.sync.dma_start(io["out_logits"][:], lg[:])
    if close:
        ctx.close()


# ---------------------------------------------------------------- build

@functools.lru_cache(maxsize=4)
def _build(stage=99):
    nc = bacc.Bacc("TRN2", target_bir_lowering=False, debug=False,
                   enable_asserts=False, num_devices=NCORE)
    io = {}

    def din(name, shape, dt):
        io[name] = nc.dram_tensor(name, list(shape), dt, kind="ExternalInput").ap()

    din("attT", [IC, TB], BF16)
    din("hidT", [512, TB], BF16)
    din("posT", [51, TB], BF16)
    din("probsP", [TBP, K], F32)
    din("sentB", [128, E], BF16)
    din("ident", [128, 128], BF16)
    din("cap_emb_bf", [CV, E], BF16)
    din("cap_emb_w_bf", [CV, E], BF16)
    din("gidx", [128, 21], mybir.dt.int32)
    din("iWT", [IC, 512], BF16)
    din("cWT", [IC, 512], BF16)
    din("hWT", [512, 256], BF16)
    din("pWT", [128, 256], BF16)
    din("posW", [51, 128], BF16)
    din("w1T", [12, 128], BF16)
    din("w2T", [384, 256], BF16)
    din("w3T", [512, 512], BF16)
    din("oWT", [128, 8], BF16)
    for d in "fb":
        din(f"wihT_{d}", [512, G3], BF16)
        din(f"whhT_{d}", [512, G3], BF16)
        din(f"gbias_{d}", [128, GC], F32)
    for nm, sz in (("ib", 512), ("pb", 256), ("hb", 256), ("cb", 512),
                   ("b1", 128), ("b2", 256), ("b3", 512)):
        din(nm, [sz], F32)
    io["out_logits"] = nc.dram_tensor("out_logits", [1, TB], F32,
                                      kind="ExternalOutput").ap()

    with tile.TileContext(nc) as tc:
        _emit(nc, tc, io, stage)
    nc.compile()
    return nc


# ---------------------------------------------------------------- host side

def _bf(x):
    return np.ascontiguousarray(np.asarray(x, np.float32).astype(BF))


def _prep_core(ci, inp, shared):
    sl = slice(ci * BPC, (ci + 1) * BPC)
    attT = _bf(np.asarray(inp["attended_img"])[sl].transpose(2, 1, 0).reshape(IC, TB))
    hidT = _bf(np.asarray(inp["hidden"])[sl].transpose(2, 1, 0).reshape(512, TB))
    posT = _bf(np.asarray(inp["pos"])[sl].transpose(2, 1, 0).reshape(51, TB))
    probsP = np.zeros((TBP, K), np.float32)
    probsP[:TB] = np.asarray(inp["topk_probs"])[:, sl, :].reshape(TB, K)
    cap = np.asarray(inp["caption"])[sl].astype(np.int64)     # [16, 21]
    tw = np.asarray(inp["topk_words"])[:, sl, :].astype(np.int64)  # [21, 16, 6]

    def pad384(a):
        o = np.zeros(TBP, np.int64)
        o[:TB] = a
        return o

    gidx = np.zeros((128, 21), np.int32)
    for k in range(K):
        col = pad384(tw[:, :, k].reshape(TB))
        for c in range(NCH):
            gidx[:, k * NCH + c] = col[c * 128:(c + 1) * 128]
    col = pad384(cap.T.reshape(TB))
    for c in range(NCH):
        gidx[:, 18 + c] = col[c * 128:(c + 1) * 128]
    m = {
        "attT": attT, "hidT": hidT, "posT": posT, "probsP": probsP,
        "sentB": np.ascontiguousarray(np.tile(shared["sent"][sl], (8, 1))),
        "gidx": gidx,
    }
    m.update(shared["weights"])
    return m


def _prep_shared(inp):
    cap_emb = np.asarray(inp["cap_embedding"], np.float32)
    capt = np.asarray(inp["caption"]).astype(np.int64)
    cap_len = np.asarray(inp["cap_len"]).astype(np.int64)
    mask = (np.arange(T)[None, :] < cap_len[:, None]).astype(np.float32)
    sent = np.einsum("bte,bt->be", cap_emb[capt], mask)       # [B, E]

    w = {}
    w["cap_emb_bf"] = _bf(cap_emb)
    w["cap_emb_w_bf"] = _bf(inp["cap_emb_w"])
    w["iWT"] = _bf(np.asarray(inp["iW"], np.float32).T)
    w["cWT"] = _bf(np.asarray(inp["cW"], np.float32).T)
    w["hWT"] = _bf(np.asarray(inp["hW"], np.float32).T)
    w["pWT"] = _bf(np.asarray(inp["pW"], np.float32).T)
    w["posW"] = _bf(inp["pos_emb_w"])
    w["w1T"] = _bf(np.asarray(inp["conv1_w"], np.float32).transpose(2, 1, 0).reshape(12, 128))
    w["w2T"] = _bf(np.asarray(inp["conv2_w"], np.float32).transpose(2, 1, 0).reshape(384, 256))
    w["w3T"] = _bf(np.asarray(inp["conv3_w"], np.float32).transpose(2, 1, 0).reshape(512, 512))
    w["oWT"] = _bf(np.asarray(inp["oW"], np.float32).T.reshape(8, 128).T)
    for d, sfx in (("f", "_f"), ("b", "_b")):
        w[f"wihT_{d}"] = _bf(np.asarray(inp["gru_w_ih" + sfx], np.float32).T)
        w[f"whhT_{d}"] = _bf(np.asarray(inp["gru_w_hh" + sfx], np.float32).T)
        gb = (np.asarray(inp["gru_b_ih" + sfx], np.float32)
              + np.asarray(inp["gru_b_hh" + sfx], np.float32))
        w[f"gbias_{d}"] = np.ascontiguousarray(gb.reshape(GC, 128).T.astype(np.float32))
    for nm, key in (("ib", "ib"), ("pb", "pb"), ("hb", "hb"), ("cb", "cb"),
                    ("b1", "conv1_b"), ("b2", "conv2_b"), ("b3", "conv3_b")):
        w[nm] = np.ascontiguousarray(np.asarray(inp[key], np.float32))
    w["ident"] = _bf(np.eye(128, dtype=np.float32))
    return {"weights": w, "sent": _bf(sent).astype(BF)}


_RUNNER = {}


def _get_runner():
    if "fn" not in _RUNNER:
        nc = _build()
        _RUNNER["nc"] = nc
        _RUNNER["fn"] = lambda in_maps: run_bass_kernel_spmd(
            nc, in_maps, core_ids=list(range(NCORE)))
    return _RUNNER["fn"]


def kernel(**inputs):
    fn = _get_runner()
    shared = _prep_shared(inputs)
    in_maps = [_prep_core(ci, inputs, shared) for ci in range(NCORE)]
    res = fn(in_maps)
    logits = np.zeros((B, T), np.float32)
    for ci in range(NCORE):
        lg = np.asarray(res.results[ci]["out_logits"], np.float32).reshape(TB)
        logits[ci * BPC:(ci + 1) * BPC] = lg.reshape(T, BPC).T
    logits += float(np.asarray(inputs["ob"]).reshape(-1)[0])
    pos = np.asarray(inputs["pos"])
    valid_pos = np.argmax(pos, axis=2) != (pos.shape[-1] - 2)
    return logits, valid_pos
